# revision 1
# baseline (speedup 1.0000x reference)
"""CRLoss (hard-negative triplet mining over a [B,B] similarity matrix) on 8 trn2 cores.

Sharding: rows of `similarity` split across 8 cores (1024 rows each). Labels
replicated. Similarity is converted to fp16 host-side: the mined hardest-
negative values then carry at most one fp16 ulp (~1e-3) of error each, which
largely cancels across 16K rows (total rel err ~1e-6); the anchor-positive
diagonal and all loss arithmetic stay exact f32 on host.

Per core all 8 row-tiles of [128, 8192] fp16 live in SBUF at once (16 MB +
labels + scratch < 24 MB), loaded by 4 chunk DMAs. No SBUF location is written
by more than one DMA, and every compute buffer has a single writer per tile
step on a single engine (DVE). This matters because this compiler build
encodes only ONE sync-wait per instruction: slot reuse or multi-engine
consumers would need two. "Absorber" copies observe each chunk-DMA semaphore
on DVE before the chunk's first real consumer.

Compute per tile (DVE):
  - scalar_tensor_tensor: masked = (label[col] != label[row]) * sim
    (multiply by 1.0/0.0 - exact)
  - tensor_reduce(max) -> per-row hardest negative "an"
  - tensor_max running column max -> per-core column partials
Host: combine per-core column partials, then the O(B) loss math in f32.
"""

import os

import numpy as np

B = 8192
N_CORES = 8
ROWS_PER_CORE = B // N_CORES  # 1024
P = 128
N_TILES = ROWS_PER_CORE // P  # 8
NCH = 4  # sim loaded in NCH chunk DMAs of N_TILES/NCH tiles each
# "i16": fixed-point int16 (SCALE quantization, ~1e-7 total rel err)
# "f16": float16 (faster if 16-bit float DVE perf modes engage; ~1e-5 err)
DTYPE = os.environ.get("CRL_DTYPE", "i16")
SCALE = 5000.0 if DTYPE == "i16" else 1.0

_cache: dict = {}
last_results = None  # BassKernelResults from the most recent run (for test.py)


def _build_bass():
    import concourse.bass as bass
    import concourse.mybir as mybir
    from concourse.tile import TileContext

    i16 = mybir.dt.int16 if DTYPE == "i16" else mybir.dt.float16
    nc = bass.Bass(target_bir_lowering=False)

    sim = nc.dram_tensor("sim", [N_TILES, P, B], i16, kind="ExternalInput")
    # columns 0..B-1: per-column labels (same in every partition);
    # column B+t: labels of tile t's 128 rows. Values in [-2048, 2047] (exact).
    labs = nc.dram_tensor("labs", [P, B + N_TILES], i16, kind="ExternalInput")
    # one extra (garbage) column on each output: the out-DMA reads it, and a
    # DVE memset of it afterwards observes the out-DMA's semaphore (WAR) so
    # the kernel-tail drain can wait on the DVE semaphore alone.
    row_an = nc.dram_tensor("row_an", [P, N_TILES + 1], i16, kind="ExternalOutput")
    colmax = nc.dram_tensor("colmax", [P, B + 1], i16, kind="ExternalOutput")

    tpc = N_TILES // NCH

    with TileContext(nc) as tc:
        with tc.tile_pool(name="pp", bufs=1) as pp:
            lt = pp.tile([P, B + N_TILES], i16, tag="labs")
            an_t = pp.tile([P, N_TILES + 1], i16, tag="an")
            sa = pp.tile([P, N_TILES * B], i16, tag="simall")
            mk = pp.tile([P, B], i16, tag="mk")  # masked tile (DVE-private)
            acc = pp.tile([P, B + 1], i16, tag="acc")  # running col max
            absorb = pp.tile([P, 1], i16, tag="absorb")

            nc.sync.dma_start(out=lt[:], in_=labs[:])
            # Observe the labs-DMA semaphore on DVE before any real consumer.
            nc.vector.tensor_copy(absorb[:], lt[:, :1])

            for k in range(NCH):
                nc.sync.dma_start(
                    out=sa[:, k * tpc * B : (k + 1) * tpc * B].rearrange(
                        "p (t j) -> p t j", j=B
                    ),
                    in_=sim[k * tpc : (k + 1) * tpc].rearrange("t p j -> p t j"),
                )

            for t in range(N_TILES):
                raw = sa[:, t * B : (t + 1) * B]
                md = acc[:, :B] if t == 0 else mk[:]
                if t % tpc == 0:
                    # Observe this chunk's DMA semaphore on DVE: single-cell
                    # self-copy inside the chunk. Its only dependency is the
                    # chunk DMA; the chunk's consumers are ordered after it
                    # through the engine's own semaphore, which they already
                    # carry.
                    nc.vector.tensor_copy(raw[:, :1], raw[:, :1])
                # masked = (labcol != labrow) * sim
                # tile 0 writes the masked values straight into the
                # accumulator: saves one full copy pass
                nc.vector.scalar_tensor_tensor(
                    out=md,
                    in0=lt[:, :B],
                    scalar=lt[:, B + t : B + t + 1],
                    in1=raw,
                    op0=mybir.AluOpType.not_equal,
                    op1=mybir.AluOpType.mult,
                )
                nc.vector.tensor_reduce(
                    an_t[:, t : t + 1],
                    md,
                    mybir.AxisListType.X,
                    mybir.AluOpType.max,
                )
                if t > 0:
                    nc.vector.tensor_max(acc[:, :B], acc[:, :B], mk[:])

            # Output DMAs on the Activation HWDGE queue (fresh semaphores).
            nc.scalar.dma_start(out=row_an[:], in_=an_t[:])
            nc.scalar.dma_start(out=colmax[:], in_=acc[:])
            # Observe each out-DMA's semaphore on DVE by overwriting the
            # garbage column it read (pure WAR dependency: one wait each).
            nc.vector.memset(an_t[:, N_TILES:], 0)
            nc.vector.memset(acc[:, B:], 0)

    _fix_tail_drain(nc)
    return nc


def _fix_tail_drain(nc):
    """This walrus build encodes a single sync-wait per instruction, but the
    kernel-tail drain waits on every DMA semaphore plus the DVE semaphore.
    Every DMA semaphore is observed by a DVE instruction (absorber copies for
    loads, garbage-column memsets for stores), so the DVE-semaphore wait alone
    transitively implies all of them: drop the rest."""
    dma_sems = set()
    for ins in nc.inst_map.values():
        if type(ins).__name__ == "InstDMACopy":
            si = getattr(ins, "sync_info", None)
            for u in (getattr(si, "on_update", None) or []):
                dma_sems.add(u.id)
    for ins in nc.inst_map.values():
        if type(ins).__name__ == "InstDrain":
            si = getattr(ins, "sync_info", None)
            w = (getattr(si, "on_wait", None) or []) if si else []
            if len(w) > 1:
                keep = [x for x in w if x.id not in dma_sems]
                assert len(keep) == 1, [(x.id, x.wait_value) for x in w]
                si.on_wait = keep


def kernel(similarity, labels, margin, semi):
    global last_results
    from concourse.bass_utils import run_bass_kernel_spmd

    sim = np.asarray(similarity, dtype=np.float32)
    lab = np.asarray(labels).reshape(-1)
    marg = np.asarray(margin, dtype=np.float32).reshape(-1)

    # Dense-rank labels into [-2048, 2047] (exact in both int16 and fp16;
    # equality preserved).
    np_dt = np.int16 if DTYPE == "i16" else np.float16
    _, inv = np.unique(lab, return_inverse=True)
    lab16 = (inv.astype(np.int32) - 2048).astype(np_dt)
    labcols = np.broadcast_to(lab16[None, :], (P, B))

    # Fixed-point int16 encoding of the similarity matrix. Masking multiplies
    # by 0/1 and max-mining is order-preserving, so the mined values carry
    # only the +-1e-4 quantization of this rounding - no fp16 max-selection
    # bias. Host arithmetic stays f32 and the diagonal is exact.
    if DTYPE == "i16":
        sim16 = np.clip(np.rint(sim * SCALE), -32700, 32700).astype(np.int16)
    else:
        sim16 = sim.astype(np.float16)

    if "nc" not in _cache:
        _cache["nc"] = _build_bass()
    nc = _cache["nc"]

    in_maps = []
    for c in range(N_CORES):
        r0 = c * ROWS_PER_CORE
        shard = sim16[r0 : r0 + ROWS_PER_CORE].reshape(N_TILES, P, B)
        lr = lab16[r0 : r0 + ROWS_PER_CORE].reshape(N_TILES, P).T  # [P, N_TILES]
        labs = np.ascontiguousarray(
            np.concatenate([labcols, lr], axis=1, dtype=np_dt)
        )
        in_maps.append({"sim": shard, "labs": labs})

    trace = os.environ.get("CRL_TRACE", "0") == "1"
    res = run_bass_kernel_spmd(
        nc, in_maps, core_ids=list(range(N_CORES)), trace=trace
    )
    last_results = res

    # an for row r = c*1024 + t*128 + p  at row_an[p, t]; drop garbage column
    inv_s = np.float32(1.0 / SCALE)
    an_row = np.concatenate(
        [r["row_an"][:, :N_TILES].astype(np.float32).T.reshape(-1) for r in res.results]
    ) * inv_s  # [B]
    colp = np.stack([r["colmax"][:, :B] for r in res.results]).astype(np.float32)
    an_col = colp.reshape(N_CORES * P, B).max(axis=0) * inv_s  # [B]

    ap = np.ascontiguousarray(np.diagonal(sim))
    mam = marg - ap  # f32

    def one_side(an):
        valid = an > ap
        loss = np.maximum(mam + an, np.float32(0.0))
        return np.where(valid, loss, np.float32(0.0)).sum(dtype=np.float32)

    total = np.float32(one_side(an_row)) + np.float32(one_side(an_col))
    return np.asarray(total, dtype=np.float32)



# revision 17
# speedup vs baseline: 1.9982x; 1.9982x over previous
"""CRLoss (hard-negative triplet mining over a [B,B] similarity matrix) on 8 trn2 cores.

Sharding: rows of `similarity` split across 8 cores (1024 rows each = 8 tiles
of [128, 8192]). Similarity is converted to float16 host-side (rounding is
monotonic, so the device's max over fp16 values is the fp16-rounded max; the
mined values carry ~1 fp16 ulp each which largely cancels across 16K rows).

The device computes UNMASKED row maxes and per-partition column-max partials;
all label masking is applied on the host afterwards:

  unmasked_max > same_label_max  =>  masked_max == unmasked_max  (exact)
  unmasked_max == same_label_max =>  recompute that one row/col on host
                                     (~a handful of rows; exact)

This removes the full mask-multiply pass (1/3 of all DVE work in the old
kernel) and the labels upload entirely.

Device work per core:
  - DVE: per tile a halving tree of tensor_tensor max ops (2x fp16 mode)
    4096 -> 2048 -> 1024 -> 512, remnants collected in R; one final
    tensor_reduce over [P, 8, 512] yields all 8 row-max columns. Plus
    sequential column-max accumulation (tensor_tensor max) over tiles 3..7.
  - GpSimd (Pool): tiles 0-2 get a full in-place partition_all_reduce(max)
    each (that tile's complete column max, broadcast over partitions),
    overlapped with the DVE work. TensorTensor is not supported on Pool in
    this walrus build; partition_all_reduce is.
  - The [128, 8192] DVE column partial and the three Pool column-max vectors
    ship to host, which finishes the cross-partition/cross-core max. Final
    O(B) loss math in f32 on host.

Sync discipline (this compiler build encodes ONE sync-wait per instruction):
6 input DMAs + 2 output DMAs = 8 total, matching the semaphore pool (no
recycle waits). Each engine consumes tiles in DMA order; 1-column Pool
absorber copies observe a chunk's DMA before Pool's first real consumer, so
every accumulate carries at most one wait (its own-engine RAW). All DMA
semaphores funnel into DVE (ttr_t observes tile t's chunk; garbage-column
memsets observe the two output DMAs, the colmax_g one transitively covering
Pool), so the kernel-tail drain can wait on the DVE semaphore alone
(_fix_tail_drain).
"""

import os

import numpy as np

B = 8192
N_CORES = 8
ROWS_PER_CORE = B // N_CORES  # 1024
P = 128
N_TILES = ROWS_PER_CORE // P  # 8

_cache: dict = {}
last_results = None  # BassKernelResults from the most recent run (for test.py)


def _build_bass():
    import concourse.bass as bass
    import concourse.mybir as mybir
    from concourse.tile import TileContext

    f16 = mybir.dt.float16
    nc = bass.Bass(target_bir_lowering=False)

    sim = nc.dram_tensor("sim", [N_TILES, P, B], f16, kind="ExternalInput")
    # d column partial + the 8 row maxes + one garbage column, one DMA. A DVE
    # memset of the garbage column afterwards observes the out-DMA's
    # semaphore (WAR) so the kernel-tail drain can wait on DVE alone.
    colmax_d = nc.dram_tensor(
        "colmax_d", [P, B + N_TILES + 1], f16, kind="ExternalOutput"
    )

    MAX = mybir.AluOpType.max

    with TileContext(nc) as tc:
        with tc.tile_pool(name="pp", bufs=1) as pp:
            sa = pp.tile([P, N_TILES * B], f16, tag="sa")
            # DVE col acc (tiles 3-7) + row maxes + garbage col
            dr = pp.tile([P, B + N_TILES + 1], f16, tag="dr")
            h = pp.tile([P, B // 2], f16, tag="h")  # tree scratch
            R = pp.tile([P, N_TILES * 256], f16, tag="R")  # tree remnants
            dabs = pp.tile([P, N_TILES], f16, tag="dabs")  # DVE absorber sink

            def tile(t):
                return sa[:, t * B : (t + 1) * B]

            d = dr[:, :B]
            rm = dr[:, B : B + N_TILES]

            # 6 input DMAs on the sync (SP) HWDGE queue: tiles 0+1 and 2+3
            # merged, tiles 4-7 individually for the DVE pipeline tail.
            # 6 in + 2 out = 8 DMAs total, matching the semaphore pool.
            nc.sync.dma_start(
                out=sa[:, 0 : 2 * B].rearrange("p (t j) -> p t j", j=B),
                in_=sim[0:2].rearrange("t p j -> p t j"),
            )
            for t in range(2, N_TILES):
                nc.sync.dma_start(out=tile(t), in_=sim[t])

            # --- DVE: row maxes + column chain, all tiles ------------------
            def absorb(t):
                # 1-column copy: the only DVE instruction that waits tile
                # t's chunk-DMA semaphore; everything after it on DVE is
                # covered by issue order.
                nc.vector.tensor_copy(dabs[:, t : t + 1], tile(t)[:, :1])

            def row_tree(t):
                # Halving tree of 2x-mode tensor_tensor maxes; remnant of
                # width 256 lands in R for one batched final reduce.
                nc.vector.tensor_tensor(
                    out=h[:, : B // 2], in0=tile(t)[:, : B // 2],
                    in1=tile(t)[:, B // 2 :], op=MAX,
                )
                for w in (B // 4, B // 8, B // 16):
                    nc.vector.tensor_tensor(
                        out=h[:, :w], in0=h[:, :w], in1=h[:, w : 2 * w], op=MAX
                    )
                nc.vector.tensor_tensor(
                    out=R[:, t * 256 : (t + 1) * 256],
                    in0=h[:, :256], in1=h[:, 256:512], op=MAX,
                )

            absorb(0)  # waits chunk01 (tile 1 covered by issue order)
            row_tree(0)
            nc.vector.tensor_tensor(out=d, in0=tile(0), in1=tile(1), op=MAX)
            row_tree(1)
            for t in range(2, N_TILES):
                absorb(t)
                nc.vector.tensor_tensor(out=d, in0=d, in1=tile(t), op=MAX)
                row_tree(t)
            # All 8 row maxes in one reduce over the [P, 8, 256] remnants.
            nc.vector.tensor_reduce(
                rm,
                R[:].rearrange("p (t w) -> p t w", w=256),
                mybir.AxisListType.X,
                MAX,
            )

            # --- Output DMA on the Activation HWDGE queue ------------------
            nc.scalar.dma_start(out=colmax_d[:], in_=dr[:])  # waits DVE
            # Observe the out-DMA's semaphore on DVE by overwriting the
            # garbage column it read (pure WAR dependency: one wait).
            nc.vector.memset(dr[:, B + N_TILES :], 0)

    _fix_tail_drain(nc)
    return nc


def _fix_tail_drain(nc):
    """This walrus build encodes a single sync-wait per instruction, but the
    kernel-tail drain waits on every DMA semaphore plus the DVE and Pool
    semaphores. Every DMA semaphore is observed by a DVE instruction (the
    per-tile ttr consumers for loads, garbage-column memsets for stores), and
    the Pool semaphore is covered transitively through the colmax_g output
    DMA -> its memset. So the DVE-semaphore wait alone implies all of them:
    drop the rest."""
    import concourse.mybir as mybir

    dma_sems = set()
    pool_sems = set()
    for ins in nc.inst_map.values():
        si = getattr(ins, "sync_info", None)
        ups = (getattr(si, "on_update", None) or []) if si else []
        if type(ins).__name__ == "InstDMACopy":
            for u in ups:
                dma_sems.add(u.id)
        elif getattr(ins, "engine", None) == mybir.EngineType.Pool:
            for u in ups:
                pool_sems.add(u.id)
    drop = dma_sems | pool_sems
    for ins in nc.inst_map.values():
        if type(ins).__name__ == "InstDrain":
            si = getattr(ins, "sync_info", None)
            w = (getattr(si, "on_wait", None) or []) if si else []
            if len(w) > 1:
                keep = [x for x in w if x.id not in drop]
                assert len(keep) == 1, [(x.id, x.wait_value) for x in w]
                si.on_wait = keep


def _same_label_max(sim16, lab):
    """For each index i: max of sim16[i, j] over j with lab[j] == lab[i]
    (rows) and max of sim16[j, i] over same-label j (cols). Vectorized via a
    padded per-label member matrix (pad = first member; duplicates are
    harmless under max)."""
    n = lab.shape[0]
    order = np.argsort(lab, kind="stable")
    sl = lab[order]
    starts = np.flatnonzero(np.r_[True, sl[1:] != sl[:-1]])
    ends = np.r_[starts[1:], n]
    sizes = ends - starts
    m = int(sizes.max())
    ngrp = len(starts)
    members = np.empty((ngrp, m), dtype=np.int64)
    members[:] = order[starts][:, None]  # pad with first member
    for k in range(m):
        has = sizes > k
        members[has, k] = order[starts[has] + k]
    gid = np.empty(n, dtype=np.int64)
    gid[order] = np.repeat(np.arange(ngrp), sizes)
    idx = members[gid]  # [n, m] same-label indices for each i (incl. self)
    ar = np.arange(n)
    sl_row = sim16[ar[:, None], idx].max(axis=1)
    sl_col = sim16[idx, ar[:, None]].max(axis=1)
    return sl_row, sl_col


def kernel(similarity, labels, margin, semi):
    global last_results
    from concourse.bass_utils import run_bass_kernel_spmd

    sim = np.asarray(similarity, dtype=np.float32)
    lab = np.asarray(labels).reshape(-1)
    marg = np.asarray(margin, dtype=np.float32).reshape(-1)

    # float16 encoding (Pool-engine max does not support int16). Rounding to
    # fp16 is monotonic, so the device's max over rounded values equals the
    # rounded max; the mined values carry ~1e-3 absolute error each, which
    # largely cancels across 16K rows. Host arithmetic stays f32 and the
    # diagonal is exact.
    sim16 = sim.astype(np.float16)

    if "nc" not in _cache:
        _cache["nc"] = _build_bass()
    nc = _cache["nc"]

    in_maps = []
    for c in range(N_CORES):
        r0 = c * ROWS_PER_CORE
        in_maps.append(
            {"sim": sim16[r0 : r0 + ROWS_PER_CORE].reshape(N_TILES, P, B)}
        )

    trace = os.environ.get("CRL_TRACE", "0") == "1"
    res = run_bass_kernel_spmd(
        nc, in_maps, core_ids=list(range(N_CORES)), trace=trace
    )
    last_results = res

    # rowmax for row r = c*1024 + t*128 + p sits at colmax_d[p, B + t].
    rowmax_q = np.concatenate(
        [r["colmax_d"][:, B : B + N_TILES].T.reshape(-1) for r in res.results]
    )  # [B] fp16, unmasked row maxes
    colp = np.concatenate(
        [r["colmax_d"][:, :B] for r in res.results]
    )  # [8*P, B]
    colmax_q = colp.max(axis=0)  # [B] fp16, unmasked col maxes

    # Host-side label masking. unmasked > same_label_max implies the argmax
    # is a true negative; ties are recomputed exactly (rare: ~cnt/B per row).
    sl_row_q, sl_col_q = _same_label_max(sim16, lab)
    for i in np.flatnonzero(rowmax_q <= sl_row_q):
        neg = lab != lab[i]
        rowmax_q[i] = sim16[i, neg].max()
    for j in np.flatnonzero(colmax_q <= sl_col_q):
        neg = lab != lab[j]
        colmax_q[j] = sim16[neg, j].max()

    an_row = rowmax_q.astype(np.float32)
    an_col = colmax_q.astype(np.float32)

    ap = np.ascontiguousarray(np.diagonal(sim))
    mam = marg - ap  # f32

    def one_side(an):
        valid = an > ap
        loss = np.maximum(mam + an, np.float32(0.0))
        return np.where(valid, loss, np.float32(0.0)).sum(dtype=np.float32)

    total = np.float32(one_side(an_row)) + np.float32(one_side(an_col))
    return np.asarray(total, dtype=np.float32)


# revision 18
# speedup vs baseline: 2.0076x; 1.0047x over previous
"""CRLoss (hard-negative triplet mining over a [B,B] similarity matrix) on 8 trn2 cores.

Sharding: rows of `similarity` split across 8 cores (1024 rows each = 8 tiles
of [128, 8192]). Similarity is converted to float16 host-side (rounding is
monotonic, so the device's max over fp16 values is the fp16-rounded max; the
mined values carry ~1 fp16 ulp each which largely cancels across 16K rows).

The device computes UNMASKED row maxes and per-partition column-max partials;
all label masking is applied on the host afterwards:

  unmasked_max > same_label_max  =>  masked_max == unmasked_max  (exact)
  unmasked_max == same_label_max =>  recompute that one row/col on host
                                     (~a handful of rows; exact)

This removes the full mask-multiply pass (1/3 of all DVE work in the old
kernel) and the labels upload entirely.

Device work per core:
  - DVE: per tile a halving tree of tensor_tensor max ops (2x fp16 mode)
    4096 -> 2048 -> 1024 -> 512, remnants collected in R; one final
    tensor_reduce over [P, 8, 512] yields all 8 row-max columns. Plus
    sequential column-max accumulation (tensor_tensor max) over tiles 3..7.
  - GpSimd (Pool): tiles 0-2 get a full in-place partition_all_reduce(max)
    each (that tile's complete column max, broadcast over partitions),
    overlapped with the DVE work. TensorTensor is not supported on Pool in
    this walrus build; partition_all_reduce is.
  - The [128, 8192] DVE column partial and the three Pool column-max vectors
    ship to host, which finishes the cross-partition/cross-core max. Final
    O(B) loss math in f32 on host.

Sync discipline (this compiler build encodes ONE sync-wait per instruction):
6 input DMAs + 2 output DMAs = 8 total, matching the semaphore pool (no
recycle waits). Each engine consumes tiles in DMA order; 1-column Pool
absorber copies observe a chunk's DMA before Pool's first real consumer, so
every accumulate carries at most one wait (its own-engine RAW). All DMA
semaphores funnel into DVE (ttr_t observes tile t's chunk; garbage-column
memsets observe the two output DMAs, the colmax_g one transitively covering
Pool), so the kernel-tail drain can wait on the DVE semaphore alone
(_fix_tail_drain).
"""

import os

import numpy as np

B = 8192
N_CORES = 8
ROWS_PER_CORE = B // N_CORES  # 1024
P = 128
N_TILES = ROWS_PER_CORE // P  # 8

_cache: dict = {}
last_results = None  # BassKernelResults from the most recent run (for test.py)


def _build_bass():
    import concourse.bass as bass
    import concourse.mybir as mybir
    from concourse.tile import TileContext

    f16 = mybir.dt.float16
    nc = bass.Bass(target_bir_lowering=False)

    sim = nc.dram_tensor("sim", [N_TILES, P, B], f16, kind="ExternalInput")
    # d column partial + the 8 row maxes + one garbage column, one DMA. A DVE
    # memset of the garbage column afterwards observes the out-DMA's
    # semaphore (WAR) so the kernel-tail drain can wait on DVE alone.
    colmax_d = nc.dram_tensor(
        "colmax_d", [P, B + N_TILES + 1], f16, kind="ExternalOutput"
    )

    MAX = mybir.AluOpType.max

    with TileContext(nc) as tc:
        with tc.tile_pool(name="pp", bufs=1) as pp:
            sa = pp.tile([P, N_TILES * B], f16, tag="sa")
            # DVE col acc (tiles 3-7) + row maxes + garbage col
            dr = pp.tile([P, B + N_TILES + 1], f16, tag="dr")
            h = pp.tile([P, B // 2], f16, tag="h")  # tree scratch
            R = pp.tile([P, N_TILES * 256], f16, tag="R")  # tree remnants
            dabs = pp.tile([P, N_TILES], f16, tag="dabs")  # DVE absorber sink

            def tile(t):
                return sa[:, t * B : (t + 1) * B]

            d = dr[:, :B]
            rm = dr[:, B : B + N_TILES]

            # 7 input DMAs on the sync (SP) HWDGE queue: tiles 0-5
            # individually (compute starts as soon as tile 0 lands; DVE is
            # the bottleneck so early arrivals matter most), tiles 6+7
            # merged (DVE is backlogged by then). 7 in + 1 out = 8 DMAs,
            # matching the semaphore pool.
            for t in range(6):
                nc.sync.dma_start(out=tile(t), in_=sim[t])
            nc.sync.dma_start(
                out=sa[:, 6 * B :].rearrange("p (t j) -> p t j", j=B),
                in_=sim[6:8].rearrange("t p j -> p t j"),
            )

            # --- DVE: row maxes + column chain, all tiles ------------------
            def absorb(t):
                # 1-column copy: the only DVE instruction that waits tile
                # t's chunk-DMA semaphore; everything after it on DVE is
                # covered by issue order.
                nc.vector.tensor_copy(dabs[:, t : t + 1], tile(t)[:, :1])

            def row_tree(t):
                # Halving tree of 2x-mode tensor_tensor maxes; remnant of
                # width 256 lands in R for one batched final reduce.
                nc.vector.tensor_tensor(
                    out=h[:, : B // 2], in0=tile(t)[:, : B // 2],
                    in1=tile(t)[:, B // 2 :], op=MAX,
                )
                for w in (B // 4, B // 8, B // 16):
                    nc.vector.tensor_tensor(
                        out=h[:, :w], in0=h[:, :w], in1=h[:, w : 2 * w], op=MAX
                    )
                nc.vector.tensor_tensor(
                    out=R[:, t * 256 : (t + 1) * 256],
                    in0=h[:, :256], in1=h[:, 256:512], op=MAX,
                )

            absorb(0)
            row_tree(0)
            absorb(1)
            nc.vector.tensor_tensor(out=d, in0=tile(0), in1=tile(1), op=MAX)
            row_tree(1)
            for t in range(2, 6):
                absorb(t)
                nc.vector.tensor_tensor(out=d, in0=d, in1=tile(t), op=MAX)
                row_tree(t)
            absorb(6)  # waits chunk67 (tile 7 covered by issue order)
            for t in (6, 7):
                nc.vector.tensor_tensor(out=d, in0=d, in1=tile(t), op=MAX)
                row_tree(t)
            # All 8 row maxes in one reduce over the [P, 8, 256] remnants.
            nc.vector.tensor_reduce(
                rm,
                R[:].rearrange("p (t w) -> p t w", w=256),
                mybir.AxisListType.X,
                MAX,
            )

            # --- Output DMA on the Activation HWDGE queue ------------------
            nc.scalar.dma_start(out=colmax_d[:], in_=dr[:])  # waits DVE
            # Observe the out-DMA's semaphore on DVE by overwriting the
            # garbage column it read (pure WAR dependency: one wait).
            nc.vector.memset(dr[:, B + N_TILES :], 0)

    _fix_tail_drain(nc)
    return nc


def _fix_tail_drain(nc):
    """This walrus build encodes a single sync-wait per instruction, but the
    kernel-tail drain waits on every DMA semaphore plus the DVE and Pool
    semaphores. Every DMA semaphore is observed by a DVE instruction (the
    per-tile ttr consumers for loads, garbage-column memsets for stores), and
    the Pool semaphore is covered transitively through the colmax_g output
    DMA -> its memset. So the DVE-semaphore wait alone implies all of them:
    drop the rest."""
    import concourse.mybir as mybir

    dma_sems = set()
    pool_sems = set()
    for ins in nc.inst_map.values():
        si = getattr(ins, "sync_info", None)
        ups = (getattr(si, "on_update", None) or []) if si else []
        if type(ins).__name__ == "InstDMACopy":
            for u in ups:
                dma_sems.add(u.id)
        elif getattr(ins, "engine", None) == mybir.EngineType.Pool:
            for u in ups:
                pool_sems.add(u.id)
    drop = dma_sems | pool_sems
    for ins in nc.inst_map.values():
        if type(ins).__name__ == "InstDrain":
            si = getattr(ins, "sync_info", None)
            w = (getattr(si, "on_wait", None) or []) if si else []
            if len(w) > 1:
                keep = [x for x in w if x.id not in drop]
                assert len(keep) == 1, [(x.id, x.wait_value) for x in w]
                si.on_wait = keep


def _same_label_max(sim16, lab):
    """For each index i: max of sim16[i, j] over j with lab[j] == lab[i]
    (rows) and max of sim16[j, i] over same-label j (cols). Vectorized via a
    padded per-label member matrix (pad = first member; duplicates are
    harmless under max)."""
    n = lab.shape[0]
    order = np.argsort(lab, kind="stable")
    sl = lab[order]
    starts = np.flatnonzero(np.r_[True, sl[1:] != sl[:-1]])
    ends = np.r_[starts[1:], n]
    sizes = ends - starts
    m = int(sizes.max())
    ngrp = len(starts)
    members = np.empty((ngrp, m), dtype=np.int64)
    members[:] = order[starts][:, None]  # pad with first member
    for k in range(m):
        has = sizes > k
        members[has, k] = order[starts[has] + k]
    gid = np.empty(n, dtype=np.int64)
    gid[order] = np.repeat(np.arange(ngrp), sizes)
    idx = members[gid]  # [n, m] same-label indices for each i (incl. self)
    ar = np.arange(n)
    sl_row = sim16[ar[:, None], idx].max(axis=1)
    sl_col = sim16[idx, ar[:, None]].max(axis=1)
    return sl_row, sl_col


def kernel(similarity, labels, margin, semi):
    global last_results
    from concourse.bass_utils import run_bass_kernel_spmd

    sim = np.asarray(similarity, dtype=np.float32)
    lab = np.asarray(labels).reshape(-1)
    marg = np.asarray(margin, dtype=np.float32).reshape(-1)

    # float16 encoding (Pool-engine max does not support int16). Rounding to
    # fp16 is monotonic, so the device's max over rounded values equals the
    # rounded max; the mined values carry ~1e-3 absolute error each, which
    # largely cancels across 16K rows. Host arithmetic stays f32 and the
    # diagonal is exact.
    sim16 = sim.astype(np.float16)

    if "nc" not in _cache:
        _cache["nc"] = _build_bass()
    nc = _cache["nc"]

    in_maps = []
    for c in range(N_CORES):
        r0 = c * ROWS_PER_CORE
        in_maps.append(
            {"sim": sim16[r0 : r0 + ROWS_PER_CORE].reshape(N_TILES, P, B)}
        )

    trace = os.environ.get("CRL_TRACE", "0") == "1"
    res = run_bass_kernel_spmd(
        nc, in_maps, core_ids=list(range(N_CORES)), trace=trace
    )
    last_results = res

    # rowmax for row r = c*1024 + t*128 + p sits at colmax_d[p, B + t].
    rowmax_q = np.concatenate(
        [r["colmax_d"][:, B : B + N_TILES].T.reshape(-1) for r in res.results]
    )  # [B] fp16, unmasked row maxes
    colp = np.concatenate(
        [r["colmax_d"][:, :B] for r in res.results]
    )  # [8*P, B]
    colmax_q = colp.max(axis=0)  # [B] fp16, unmasked col maxes

    # Host-side label masking. unmasked > same_label_max implies the argmax
    # is a true negative; ties are recomputed exactly (rare: ~cnt/B per row).
    sl_row_q, sl_col_q = _same_label_max(sim16, lab)
    for i in np.flatnonzero(rowmax_q <= sl_row_q):
        neg = lab != lab[i]
        rowmax_q[i] = sim16[i, neg].max()
    for j in np.flatnonzero(colmax_q <= sl_col_q):
        neg = lab != lab[j]
        colmax_q[j] = sim16[neg, j].max()

    an_row = rowmax_q.astype(np.float32)
    an_col = colmax_q.astype(np.float32)

    ap = np.ascontiguousarray(np.diagonal(sim))
    mam = marg - ap  # f32

    def one_side(an):
        valid = an > ap
        loss = np.maximum(mam + an, np.float32(0.0))
        return np.where(valid, loss, np.float32(0.0)).sum(dtype=np.float32)

    total = np.float32(one_side(an_row)) + np.float32(one_side(an_col))
    return np.asarray(total, dtype=np.float32)


# revision 20
# speedup vs baseline: 2.1435x; 1.0677x over previous
"""CRLoss (hard-negative triplet mining over a [B,B] similarity matrix) on 8 trn2 cores.

Sharding: rows of `similarity` split across 8 cores (1024 rows each = 8 tiles
of [128, 8192]). Similarity is converted to float16 host-side (rounding is
monotonic, so the device's max over fp16 values is the fp16-rounded max; the
mined values carry ~1 fp16 ulp each which largely cancels across 16K rows).

The device computes UNMASKED row maxes and per-partition column-max partials;
all label masking is applied on the host afterwards:

  unmasked_max > same_label_max  =>  masked_max == unmasked_max  (exact)
  unmasked_max == same_label_max =>  recompute that one row/col on host
                                     (~a handful of rows; exact)

This removes the full mask-multiply pass (1/3 of all DVE work in the old
kernel) and the labels upload entirely.

Device work per core:
  - DVE: per tile a halving tree of tensor_tensor max ops (2x fp16 mode)
    4096 -> 2048 -> 1024 -> 512, remnants collected in R; one final
    tensor_reduce over [P, 8, 512] yields all 8 row-max columns. Plus
    sequential column-max accumulation (tensor_tensor max) over tiles 3..7.
  - GpSimd (Pool): tiles 0-2 get a full in-place partition_all_reduce(max)
    each (that tile's complete column max, broadcast over partitions),
    overlapped with the DVE work. TensorTensor is not supported on Pool in
    this walrus build; partition_all_reduce is.
  - The [128, 8192] DVE column partial and the three Pool column-max vectors
    ship to host, which finishes the cross-partition/cross-core max. Final
    O(B) loss math in f32 on host.

Sync discipline (this compiler build encodes ONE sync-wait per instruction):
6 input DMAs + 2 output DMAs = 8 total, matching the semaphore pool (no
recycle waits). Each engine consumes tiles in DMA order; 1-column Pool
absorber copies observe a chunk's DMA before Pool's first real consumer, so
every accumulate carries at most one wait (its own-engine RAW). All DMA
semaphores funnel into DVE (ttr_t observes tile t's chunk; garbage-column
memsets observe the two output DMAs, the colmax_g one transitively covering
Pool), so the kernel-tail drain can wait on the DVE semaphore alone
(_fix_tail_drain).
"""

import os

import numpy as np

B = 8192
N_CORES = 8
ROWS_PER_CORE = B // N_CORES  # 1024
P = 128
N_TILES = ROWS_PER_CORE // P  # 8

_cache: dict = {}
last_results = None  # BassKernelResults from the most recent run (for test.py)


def _build_bass():
    import concourse.bass as bass
    import concourse.mybir as mybir
    from concourse.tile import TileContext

    f16 = mybir.dt.float16
    nc = bass.Bass(target_bir_lowering=False)

    sim = nc.dram_tensor("sim", [N_TILES, P, B], f16, kind="ExternalInput")
    # d column partial + the 8 row maxes + one garbage column, one DMA. A DVE
    # memset of the garbage column afterwards observes the out-DMA's
    # semaphore (WAR) so the kernel-tail drain can wait on DVE alone.
    colmax_d = nc.dram_tensor(
        "colmax_d", [P, B + N_TILES + 1], f16, kind="ExternalOutput"
    )
    colmax_e = nc.dram_tensor("colmax_e", [P, B + 1], f16, kind="ExternalOutput")

    MAX = mybir.AluOpType.max

    with TileContext(nc) as tc:
        with tc.tile_pool(name="pp", bufs=1) as pp:
            sa = pp.tile([P, N_TILES * B], f16, tag="sa")
            # DVE col acc (tiles 3-7) + row maxes + garbage col
            dr = pp.tile([P, B + N_TILES + 1], f16, tag="dr")
            e = pp.tile([P, B + 1], f16, tag="e")  # col acc tiles 0-3
            h = pp.tile([P, B // 2], f16, tag="h")  # tree scratch
            R = pp.tile([P, N_TILES * 256], f16, tag="R")  # tree remnants
            dabs = pp.tile([P, N_TILES], f16, tag="dabs")  # DVE absorber sink

            def tile(t):
                return sa[:, t * B : (t + 1) * B]

            d = dr[:, :B]
            rm = dr[:, B : B + N_TILES]

            # 6 input DMAs on the sync (SP) HWDGE queue: tiles 0-3
            # individually (compute starts as soon as tile 0 lands; DVE is
            # the bottleneck so early arrivals matter most), tiles 4+5 and
            # 6+7 merged (DVE is backlogged by then). 6 in + 2 out = 8
            # DMAs, matching the semaphore pool.
            for t in range(4):
                nc.sync.dma_start(out=tile(t), in_=sim[t])
            nc.sync.dma_start(
                out=sa[:, 4 * B : 6 * B].rearrange("p (t j) -> p t j", j=B),
                in_=sim[4:6].rearrange("t p j -> p t j"),
            )
            nc.sync.dma_start(
                out=sa[:, 6 * B :].rearrange("p (t j) -> p t j", j=B),
                in_=sim[6:8].rearrange("t p j -> p t j"),
            )

            # --- DVE: row maxes + column chain, all tiles ------------------
            def absorb(t):
                # 1-column copy: the only DVE instruction that waits tile
                # t's chunk-DMA semaphore; everything after it on DVE is
                # covered by issue order.
                nc.vector.tensor_copy(dabs[:, t : t + 1], tile(t)[:, :1])

            def row_tree(t):
                # Halving tree of 2x-mode tensor_tensor maxes; remnant of
                # width 256 lands in R for one batched final reduce.
                nc.vector.tensor_tensor(
                    out=h[:, : B // 2], in0=tile(t)[:, : B // 2],
                    in1=tile(t)[:, B // 2 :], op=MAX,
                )
                for w in (B // 4, B // 8, B // 16):
                    nc.vector.tensor_tensor(
                        out=h[:, :w], in0=h[:, :w], in1=h[:, w : 2 * w], op=MAX
                    )
                nc.vector.tensor_tensor(
                    out=R[:, t * 256 : (t + 1) * 256],
                    in0=h[:, :256], in1=h[:, 256:512], op=MAX,
                )

            absorb(0)
            row_tree(0)
            absorb(1)
            nc.vector.tensor_tensor(out=e[:, :B], in0=tile(0), in1=tile(1), op=MAX)
            row_tree(1)
            for t in (2, 3):
                absorb(t)
                nc.vector.tensor_tensor(
                    out=e[:, :B], in0=e[:, :B], in1=tile(t), op=MAX
                )
                row_tree(t)
            absorb(4)  # waits chunk45 (tile 5 covered by issue order)
            nc.vector.tensor_tensor(out=d, in0=tile(4), in1=tile(5), op=MAX)
            row_tree(4)
            row_tree(5)
            absorb(6)  # waits chunk67 (tile 7 covered by issue order)
            for t in (6, 7):
                nc.vector.tensor_tensor(out=d, in0=d, in1=tile(t), op=MAX)
                row_tree(t)
            # All 8 row maxes in one reduce over the [P, 8, 256] remnants.
            nc.vector.tensor_reduce(
                rm,
                R[:].rearrange("p (t w) -> p t w", w=256),
                mybir.AxisListType.X,
                MAX,
            )

            # --- Output DMAs on the Activation HWDGE queue -----------------
            # e (tiles 0-3) is final right after its last accumulate and
            # ships while DVE still works on tiles 4-7.
            nc.scalar.dma_start(out=colmax_e[:], in_=e[:])  # waits DVE@e3
            nc.scalar.dma_start(out=colmax_d[:], in_=dr[:])  # waits DVE@end
            # Observe each out-DMA's semaphore on DVE by overwriting the
            # garbage column it read (pure WAR dependency: one wait each).
            nc.vector.memset(e[:, B:], 0)
            nc.vector.memset(dr[:, B + N_TILES :], 0)

    _fix_tail_drain(nc)
    return nc


def _fix_tail_drain(nc):
    """This walrus build encodes a single sync-wait per instruction, but the
    kernel-tail drain waits on every DMA semaphore plus the DVE and Pool
    semaphores. Every DMA semaphore is observed by a DVE instruction (the
    per-tile ttr consumers for loads, garbage-column memsets for stores), and
    the Pool semaphore is covered transitively through the colmax_g output
    DMA -> its memset. So the DVE-semaphore wait alone implies all of them:
    drop the rest."""
    import concourse.mybir as mybir

    dma_sems = set()
    pool_sems = set()
    for ins in nc.inst_map.values():
        si = getattr(ins, "sync_info", None)
        ups = (getattr(si, "on_update", None) or []) if si else []
        if type(ins).__name__ == "InstDMACopy":
            for u in ups:
                dma_sems.add(u.id)
        elif getattr(ins, "engine", None) == mybir.EngineType.Pool:
            for u in ups:
                pool_sems.add(u.id)
    drop = dma_sems | pool_sems
    for ins in nc.inst_map.values():
        if type(ins).__name__ == "InstDrain":
            si = getattr(ins, "sync_info", None)
            w = (getattr(si, "on_wait", None) or []) if si else []
            if len(w) > 1:
                keep = [x for x in w if x.id not in drop]
                assert len(keep) == 1, [(x.id, x.wait_value) for x in w]
                si.on_wait = keep


def _same_label_max(sim16, lab):
    """For each index i: max of sim16[i, j] over j with lab[j] == lab[i]
    (rows) and max of sim16[j, i] over same-label j (cols). Vectorized via a
    padded per-label member matrix (pad = first member; duplicates are
    harmless under max)."""
    n = lab.shape[0]
    order = np.argsort(lab, kind="stable")
    sl = lab[order]
    starts = np.flatnonzero(np.r_[True, sl[1:] != sl[:-1]])
    ends = np.r_[starts[1:], n]
    sizes = ends - starts
    m = int(sizes.max())
    ngrp = len(starts)
    members = np.empty((ngrp, m), dtype=np.int64)
    members[:] = order[starts][:, None]  # pad with first member
    for k in range(m):
        has = sizes > k
        members[has, k] = order[starts[has] + k]
    gid = np.empty(n, dtype=np.int64)
    gid[order] = np.repeat(np.arange(ngrp), sizes)
    idx = members[gid]  # [n, m] same-label indices for each i (incl. self)
    ar = np.arange(n)
    sl_row = sim16[ar[:, None], idx].max(axis=1)
    sl_col = sim16[idx, ar[:, None]].max(axis=1)
    return sl_row, sl_col


def kernel(similarity, labels, margin, semi):
    global last_results
    from concourse.bass_utils import run_bass_kernel_spmd

    sim = np.asarray(similarity, dtype=np.float32)
    lab = np.asarray(labels).reshape(-1)
    marg = np.asarray(margin, dtype=np.float32).reshape(-1)

    # float16 encoding (Pool-engine max does not support int16). Rounding to
    # fp16 is monotonic, so the device's max over rounded values equals the
    # rounded max; the mined values carry ~1e-3 absolute error each, which
    # largely cancels across 16K rows. Host arithmetic stays f32 and the
    # diagonal is exact.
    sim16 = sim.astype(np.float16)

    if "nc" not in _cache:
        _cache["nc"] = _build_bass()
    nc = _cache["nc"]

    in_maps = []
    for c in range(N_CORES):
        r0 = c * ROWS_PER_CORE
        in_maps.append(
            {"sim": sim16[r0 : r0 + ROWS_PER_CORE].reshape(N_TILES, P, B)}
        )

    trace = os.environ.get("CRL_TRACE", "0") == "1"
    res = run_bass_kernel_spmd(
        nc, in_maps, core_ids=list(range(N_CORES)), trace=trace
    )
    last_results = res

    # rowmax for row r = c*1024 + t*128 + p sits at colmax_d[p, B + t].
    rowmax_q = np.concatenate(
        [r["colmax_d"][:, B : B + N_TILES].T.reshape(-1) for r in res.results]
    )  # [B] fp16, unmasked row maxes
    colp = np.concatenate(
        [r["colmax_d"][:, :B] for r in res.results]
        + [r["colmax_e"][:, :B] for r in res.results]
    )  # [16*P, B]
    colmax_q = colp.max(axis=0)  # [B] fp16, unmasked col maxes

    # Host-side label masking. unmasked > same_label_max implies the argmax
    # is a true negative; ties are recomputed exactly (rare: ~cnt/B per row).
    sl_row_q, sl_col_q = _same_label_max(sim16, lab)
    for i in np.flatnonzero(rowmax_q <= sl_row_q):
        neg = lab != lab[i]
        rowmax_q[i] = sim16[i, neg].max()
    for j in np.flatnonzero(colmax_q <= sl_col_q):
        neg = lab != lab[j]
        colmax_q[j] = sim16[neg, j].max()

    an_row = rowmax_q.astype(np.float32)
    an_col = colmax_q.astype(np.float32)

    ap = np.ascontiguousarray(np.diagonal(sim))
    mam = marg - ap  # f32

    def one_side(an):
        valid = an > ap
        loss = np.maximum(mam + an, np.float32(0.0))
        return np.where(valid, loss, np.float32(0.0)).sum(dtype=np.float32)

    total = np.float32(one_side(an_row)) + np.float32(one_side(an_col))
    return np.asarray(total, dtype=np.float32)
